# revision 1
# baseline (speedup 1.0000x reference)
"""2-layer GCN on 8 TRN2 cores — v2.

v2 changes vs v1:
  - S (one-hot x norm, bf16) is built on HOST and DMA-loaded per chunk
    (kills the DVE is_equal expansion that dominated v1).
  - Phase D gathers from a device-built h2rep[50000, 64] f32 table (each row =
    h2[n] replicated), indexed by the SAME LO/HI src indices as phase A
    (idxA reused verbatim); extraction is a strided copy, no masking.
  - PSUM->SBUF evictions moved to the (idle) Activation engine.
"""
import math
import numpy as np
import ml_dtypes

import concourse.bass as bass
import concourse.bacc as bacc
import concourse.tile as tile
import concourse.mybir as mybir
from concourse import library_config

P = 128
REPW = 64         # h2rep row width (f32 -> 256B)
NBLK = 4          # dst blocks per chunk
MAX_DESC = 1024   # single-packet dma_gather limit
D_SINGLE_PACKET = True
D_BARRIER = True


def _build_side(idx_val, is_lo, core2, blk2, dloc2, n_cores, nb):
    """Generic slot/chunk/instruction structure for one gather side.

    idx_val: rebased gather index per edge (int, already valid for its group's
    table slice). is_lo: group flag. All arrays sorted by (core, blk, ~is_lo).
    Returns dict(chunks, tot, instrs, idx[], S[]).
    """
    counts = np.zeros((n_cores, nb, 2), np.int64)
    np.add.at(counts, (core2, blk2, (~is_lo).astype(np.int64)), 1)
    kL = np.ceil(counts[:, :, 0] / P).astype(np.int64).max(axis=0)
    kH = np.ceil(counts[:, :, 1] / P).astype(np.int64).max(axis=0)

    chunks = []
    slot_ptr = 0
    for c0 in range(0, nb, NBLK):
        blocks = list(range(c0, min(c0 + NBLK, nb)))
        lo_runs = {}
        lo_start = slot_ptr
        for b in blocks:
            lo_runs[b] = (slot_ptr, slot_ptr + int(kL[b]))
            slot_ptr += int(kL[b])
        lo_end = slot_ptr
        hi_runs = {}
        for b in blocks:
            hi_runs[b] = (slot_ptr, slot_ptr + int(kH[b]))
            slot_ptr += int(kH[b])
        chunks.append(dict(blocks=blocks, lo=(lo_start, lo_end),
                           hi=(lo_end, slot_ptr), lo_runs=lo_runs,
                           hi_runs=hi_runs))
    tot = slot_ptr

    idx_f = np.zeros((n_cores, tot * P), np.int16)
    # dst_local per edge slot-position; 255 = padding (matches no one-hot col)
    dl_f = np.full((n_cores, tot * P), 255.0, np.float32)

    key = (core2 * nb + blk2) * 2 + (~is_lo).astype(np.int64)
    run_start = np.searchsorted(key, np.arange(n_cores * nb * 2))
    nE = len(core2)
    for c in range(n_cores):
        for b in range(nb):
            ch = chunks[b // NBLK]
            for g, runs in ((0, ch["lo_runs"]), (1, ch["hi_runs"])):
                kidx = (c * nb + b) * 2 + g
                a = run_start[kidx]
                e = run_start[kidx + 1] if kidx + 1 < len(run_start) else nE
                n = e - a
                if n == 0:
                    continue
                s0, s1 = runs[b]
                assert n <= (s1 - s0) * P
                fl = np.arange(n) + s0 * P
                idx_f[c, fl] = idx_val[a:e].astype(np.int16)
                dl_f[c, fl] = dloc2[a:e]

    def wrap(flat_i16, s0, s1):
        seg = flat_i16[s0 * P:s1 * P]
        return np.tile(seg.reshape(16, -1, order="F"), (8, 1))

    instrs0 = []
    for ch in chunks:
        if ch["lo"][1] > ch["lo"][0]:
            instrs0.append(ch["lo"])
        if ch["hi"][1] > ch["hi"][0]:
            instrs0.append(ch["hi"])
    instrs = []
    for s0, s1 in instrs0:
        n = s1 - s0
        parts = math.ceil(n * P / MAX_DESC)
        step = math.ceil(n / parts)
        for a in range(s0, s1, step):
            instrs.append((a, min(a + step, s1)))

    idx = [np.concatenate([wrap(idx_f[c], s0, s1) for (s0, s1) in instrs], axis=1)
           for c in range(n_cores)]
    # host-built one-hot S [P, tot*P] bf16 (pure 0/1; dinv factored out)
    S_host = []
    for c in range(n_cores):
        S = np.zeros((P, tot * P), ml_dtypes.bfloat16)
        dl = dl_f[c]
        valid = dl < P
        fl = np.nonzero(valid)[0]
        S[fl % P, (fl // P) * P + dl[fl].astype(np.int64)] = 1.0
        S_host.append(S)
    return dict(chunks=chunks, tot=tot, instrs=instrs, idx=idx, S=S_host)


def make_plan(edge_index, n_nodes, n_cores, lo_split=32768):
    src = np.concatenate([edge_index[0], np.arange(n_nodes)]).astype(np.int64)
    dst = np.concatenate([edge_index[1], np.arange(n_nodes)]).astype(np.int64)
    deg = np.bincount(dst, minlength=n_nodes).astype(np.float32)
    dinv = 1.0 / np.sqrt(np.maximum(deg, 1.0))
    norm = (dinv[src] * dinv[dst]).astype(np.float32)

    n_own = n_nodes // n_cores
    assert n_own * n_cores == n_nodes
    nb = math.ceil(n_own / P)
    n_pad = nb * P

    core = (dst // n_own).astype(np.int64)
    loc = dst - core * n_own

    # balanced dst->block assignment (snake-deal by degree)
    blk_of = np.zeros((n_cores, n_own), np.int64)
    dloc_of = np.zeros((n_cores, n_own), np.int64)
    degc = np.zeros((n_cores, n_own), np.int64)
    np.add.at(degc, (core, loc), 1)
    snake = np.concatenate([np.arange(nb), np.arange(nb)[::-1]])
    for c in range(n_cores):
        order = np.argsort(-degc[c], kind="stable")
        bseq = snake[np.arange(n_own) % (2 * nb)]
        blk_of[c, order] = bseq
        pos = np.zeros(nb, np.int64)
        dl = np.zeros(n_own, np.int64)
        for i, d in enumerate(order):
            b = bseq[i]
            dl[d] = pos[b]
            pos[b] += 1
        assert pos.max() <= P
        dloc_of[c] = dl
    blk = blk_of[core, loc]
    dloc = dloc_of[core, loc]

    # h2all position of each node (phase D gather index space)
    src_core = src // n_own
    src_loc = src - src_core * n_own
    h2pos = (src_core * n_pad + blk_of[src_core, src_loc] * P
             + dloc_of[src_core, src_loc])

    # ---------- phase A side (split by src id) ----------
    is_loA = src < lo_split
    orderA = np.lexsort((~is_loA, blk, core))
    vA = np.where(is_loA, src, src - lo_split)[orderA]
    sideA = _build_side(vA, is_loA[orderA], core[orderA], blk[orderA],
                        dloc[orderA], n_cores, nb)

    # ---------- phase D side (split by h2all position) ----------
    is_loD = h2pos < lo_split
    orderD = np.lexsort((~is_loD, blk, core))
    vD = np.where(is_loD, h2pos, h2pos - lo_split)[orderD]
    sideD = _build_side(vD, is_loD[orderD], core[orderD], blk[orderD],
                        dloc[orderD], n_cores, nb)

    # per-core dst-side dinv tables (in permuted block layout)
    n_rep = n_cores * n_pad
    dinv_pos = np.zeros(n_rep, np.float32)        # dinv by h2all position
    allpos = (np.repeat(np.arange(n_cores), n_own) * n_pad
              + (blk_of * P + dloc_of).reshape(-1))
    dinv_pos[allpos] = dinv[np.arange(n_nodes)]
    dinvrep = []                                  # [P, n_pad] row-replicated
    dinv_cols = []                                # [P, nb] column per block
    for c in range(n_cores):
        dp = dinv_pos[c * n_pad:(c + 1) * n_pad]
        dinvrep.append(np.tile(dp, (P, 1)).astype(np.float32))
        dinv_cols.append(np.ascontiguousarray(
            dp.reshape(nb, P).T.astype(np.float32)))
    return dict(
        n_own=n_own, nb=nb, n_pad=n_pad, lo_split=lo_split,
        blk_of=blk_of, dloc_of=dloc_of, dinv=dinv, dinv_pos=dinv_pos,
        dinvrep=dinvrep, dinv_cols=dinv_cols,
        tot=sideA["tot"], chunks=sideA["chunks"], instrs_a=sideA["instrs"],
        idxA=sideA["idx"], SA=sideA["S"],
        tot_d=sideD["tot"], chunks_d=sideD["chunks"], instrs_d=sideD["instrs"],
        idxD=sideD["idx"], SD=sideD["S"],
    )


def build_kernel(plan, n_nodes, n_cores, npass=1, phases="abcd"):
    nb, tot, n_pad = plan["nb"], plan["tot"], plan["n_pad"]
    n_own = plan["n_own"]
    tot_d = plan["tot_d"]
    lo_split = plan["lo_split"]
    n_rep = n_cores * n_pad               # h2rep rows (h2all positions)
    RP = next(rp for rp in (128, 125, 100, 64, 50, 25, 16, 8)
              if n_rep % rp == 0)
    RK = n_rep // RP                      # h2rep SBUF staging shape

    nc = bacc.Bacc("TRN2", target_bir_lowering=False, debug=False,
                   enable_asserts=False, num_devices=n_cores, num_swdge_queues=4)
    f32, bf16, i16 = mybir.dt.float32, mybir.dt.bfloat16, mybir.dt.int16

    xt = nc.dram_tensor("xt", [n_nodes, P], bf16, kind="ExternalInput").ap()
    idxA = nc.dram_tensor("idxA", [P, tot * 8], i16, kind="ExternalInput").ap()
    idxD = nc.dram_tensor("idxD", [P, tot_d * 8], i16, kind="ExternalInput").ap()
    Sd = nc.dram_tensor("Sd", [P, tot * P], bf16, kind="ExternalInput").ap()
    S2d = nc.dram_tensor("S2d", [P, tot_d * P], bf16, kind="ExternalInput").ap()
    dinvrep = nc.dram_tensor("dinvrep", [P, n_pad], f32, kind="ExternalInput").ap()
    dinvcol = nc.dram_tensor("dinvcol", [P, nb], f32, kind="ExternalInput").ap()
    dinvpos = nc.dram_tensor("dinvpos", [n_cores * n_pad, 1], f32,
                             kind="ExternalInput").ap()
    w1 = nc.dram_tensor("w1", [P, P], bf16, kind="ExternalInput").ap()
    b1 = nc.dram_tensor("b1", [P, 1], f32, kind="ExternalInput").ap()
    w2 = nc.dram_tensor("w2", [P, 1], bf16, kind="ExternalInput").ap()
    b2 = nc.dram_tensor("b2", [P, 1], f32, kind="ExternalInput").ap()
    out = nc.dram_tensor("out", [n_pad, 1], f32, kind="ExternalOutput").ap()

    qn = [0]

    def next_q():
        qn[0] = (qn[0] + 1) % 4
        return qn[0]

    with tile.TileContext(nc) as tc:
        with (
            tc.tile_pool(name="const", bufs=1) as cpool,
            tc.tile_pool(name="dram", bufs=max(npass, 1), space="DRAM") as dpool,
        ):
            nc.gpsimd.load_library(library_config.mlp)
            idxA_t = cpool.tile([P, tot * 8], i16)
            idxD_t = cpool.tile([P, tot_d * 8], i16)
            dinvrep_t = cpool.tile([P, n_pad], f32)
            dinvcol_t = cpool.tile([P, nb], f32)
            w1_t = cpool.tile([P, P], bf16)
            b1_t = cpool.tile([P, 1], f32)
            w2_t = cpool.tile([P, 1], bf16)
            b2_t = cpool.tile([P, 1], f32)
            h2strip = cpool.tile([1, n_pad], f32)
            for t, d in ((idxA_t, idxA), (idxD_t, idxD), (dinvrep_t, dinvrep),
                         (dinvcol_t, dinvcol), (w1_t, w1),
                         (b1_t, b1), (w2_t, w2), (b2_t, b2)):
                nc.sync.dma_start(t[:], d[:])

            for _ps in range(npass):
                if D_BARRIER and _ps:
                    tc.strict_bb_all_engine_barrier()
                h2loc = dpool.tile([1, n_pad], f32, name=f"h2loc{_ps}")
                h2all = dpool.tile([n_cores, n_pad], f32, addr_space="Shared",
                                   name=f"h2all{_ps}")
                h2rep = dpool.tile([n_rep, REPW], f32, name=f"h2rep{_ps}")

                # ---------------- phase A + B ----------------
                with (
                    tc.tile_pool(name="gbuf", bufs=2) as gpool,
                    tc.tile_pool(name="spool", bufs=2) as spool,
                    tc.tile_pool(name="small_ab", bufs=2) as smpool,
                    tc.tile_pool(name="agg_ps", bufs=3, space="PSUM") as agg_ps,
                    tc.tile_pool(name="o1_ps", bufs=2, space="PSUM") as o1_ps,
                    tc.tile_pool(name="h2_ps", bufs=2, space="PSUM") as h2_ps,
                ):
                    ia = 0
                    icol = 0
                    for ch in plan["chunks"]:
                        c0, c1 = ch["lo"][0], ch["hi"][1]
                        k = c1 - c0
                        G = gpool.tile([P, k * P], bf16, tag="G", name="G")
                        while ia < len(plan["instrs_a"]) and plan["instrs_a"][ia][0] < c1:
                            s0, s1 = plan["instrs_a"][ia]
                            n = s1 - s0
                            src_tab = (xt[:lo_split, :] if s0 < ch["hi"][0]
                                       else xt[lo_split:, :])
                            nc.gpsimd.dma_gather(
                                G[:, (s0 - c0) * P:(s1 - c0) * P].rearrange(
                                    "p (c d) -> p c d", d=P),
                                src_tab,
                                idxA_t[:, icol:icol + n * 8],
                                n * P, n * P, P,
                                single_packet=True, queue_num=next_q(),
                            )
                            icol += n * 8
                            ia += 1

                        S = spool.tile([P, k * P], bf16, tag="S", name="S")
                        eng = nc.scalar if (c0 // NBLK) % 2 else nc.sync
                        eng.dma_start(S[:], Sd[:, c0 * P:c1 * P])

                        for b in ch["blocks"]:
                            slots = (list(range(*ch["lo_runs"][b]))
                                     + list(range(*ch["hi_runs"][b])))
                            aggT = agg_ps.tile([P, P], f32, tag="agg", name="aggT")
                            for i, s in enumerate(slots):
                                sl = slice((s - c0) * P, (s - c0 + 1) * P)
                                nc.tensor.matmul(
                                    out=aggT[:], lhsT=G[:, sl], rhs=S[:, sl],
                                    start=(i == 0), stop=(i == len(slots) - 1))
                            aggT_sb = smpool.tile([P, P], bf16, tag="aggsb",
                                                  name="aggT_sb")
                            nc.vector.tensor_tensor(
                                out=aggT_sb[:], in0=aggT[:],
                                in1=dinvrep_t[:, b * P:(b + 1) * P],
                                op=mybir.AluOpType.mult)
                            o1 = o1_ps.tile([P, P], f32, tag="o1", name="o1")
                            nc.tensor.matmul(out=o1[:], lhsT=w1_t[:], rhs=aggT_sb[:],
                                             start=True, stop=True)
                            r1 = smpool.tile([P, P], bf16, tag="r1", name="r1")
                            nc.scalar.activation(r1[:], o1[:],
                                                 mybir.ActivationFunctionType.Relu,
                                                 bias=b1_t[:, 0:1])
                            h2p = h2_ps.tile([1, P], f32, tag="h2", name="h2p")
                            nc.tensor.matmul(out=h2p[:], lhsT=w2_t[:], rhs=r1[:],
                                             start=True, stop=True)
                            nc.vector.tensor_copy(
                                h2strip[0:1, b * P:(b + 1) * P], h2p[:])

                # ---------------- phase C ----------------
                if "c" not in phases:
                    nc.sync.dma_start(out[0:P, 0:1],
                                      h2strip[0:1, 0:P].rearrange("a b -> b a"))
                    continue
                if D_BARRIER:
                    tc.strict_bb_all_engine_barrier()
                nc.sync.dma_start(h2loc[:], h2strip[:])
                nc.gpsimd.collective_compute(
                    "AllGather", mybir.AluOpType.bypass,
                    ins=[h2loc.opt()], outs=[h2all.opt()],
                    replica_groups=[list(range(n_cores))],
                )
                # build h2rep [n_nodes, REPW]
                with tc.tile_pool(name="rep", bufs=1) as rpool:
                    hin = rpool.tile([RP, RK], f32)
                    nc.sync.dma_start(
                        hin[:], h2all[:].rearrange("a b -> (a b)").rearrange(
                            "(p k) -> p k", p=RP))
                    dv = rpool.tile([RP, RK], f32)
                    nc.sync.dma_start(
                        dv[:], dinvpos[:, 0:1].rearrange("(p k) a -> p (k a)", p=RP))
                    hsc = rpool.tile([RP, RK], f32)
                    nc.vector.tensor_tensor(out=hsc[:], in0=hin[:], in1=dv[:],
                                            op=mybir.AluOpType.mult)
                    hx = rpool.tile([RP, RK * REPW], f32)
                    nc.vector.tensor_copy(
                        hx[:].rearrange("p (k d) -> p k d", d=REPW),
                        hsc[:].rearrange("p (k a) -> p k a", a=1).to_broadcast(
                            [RP, RK, REPW]))
                    nc.sync.dma_start(
                        h2rep[:].rearrange("(p k) d -> p (k d)", p=RP), hx[:])

                # ---------------- phase D ----------------
                if "d" not in phases:
                    nc.sync.dma_start(out[0:P, 0:1],
                                      h2strip[0:1, 0:P].rearrange("a b -> b a"))
                    continue
                with (
                    tc.tile_pool(name="repg", bufs=2) as repg,
                    tc.tile_pool(name="s2pool", bufs=2) as s2pool,
                    tc.tile_pool(name="smp2", bufs=2) as smp2,
                    tc.tile_pool(name="o2_ps", bufs=2, space="PSUM") as o2_ps,
                ):
                    ia = 0
                    icol = 0
                    for ch in plan["chunks_d"]:
                        c0, c1 = ch["lo"][0], ch["hi"][1]
                        k = c1 - c0
                        R = repg.tile([P, k * REPW], f32, tag="R", name="R")
                        while ia < len(plan["instrs_d"]) and plan["instrs_d"][ia][0] < c1:
                            s0, s1 = plan["instrs_d"][ia]
                            n = s1 - s0
                            rep_tab = (h2rep[:lo_split, :] if s0 < ch["hi"][0]
                                       else h2rep[lo_split:, :])
                            nc.gpsimd.dma_gather(
                                R[:, (s0 - c0) * REPW:(s1 - c0) * REPW].rearrange(
                                    "p (c d) -> p c d", d=REPW),
                                rep_tab,
                                idxD_t[:, icol:icol + n * 8],
                                n * P, n * P, REPW,
                                single_packet=D_SINGLE_PACKET, queue_num=next_q(),
                            )
                            icol += n * 8
                            ia += 1

                        S = s2pool.tile([P, k * P], bf16, tag="S2", name="S2")
                        eng = nc.sync if (c0 // NBLK) % 2 else nc.scalar
                        eng.dma_start(S[:], S2d[:, c0 * P:c1 * P])
                        h2sb = smp2.tile([P, k], bf16, tag="h2s", name="h2sb")
                        nc.vector.tensor_copy(
                            h2sb[:],
                            R[:].rearrange("p (c d) -> p c d", d=REPW)[:, :, 0:1])

                        for b in ch["blocks"]:
                            slots = (list(range(*ch["lo_runs"][b]))
                                     + list(range(*ch["hi_runs"][b])))
                            o2 = o2_ps.tile([P, 1], f32, tag="o2", name="o2")
                            for i, s in enumerate(slots):
                                sl = slice((s - c0) * P, (s - c0 + 1) * P)
                                nc.tensor.matmul(
                                    out=o2[:], lhsT=S[:, sl],
                                    rhs=h2sb[:, (s - c0):(s - c0 + 1)],
                                    start=(i == 0), stop=(i == len(slots) - 1))
                            osb = smp2.tile([P, 1], f32, tag="osb", name="osb")
                            nc.scalar.activation(
                                osb[:], o2[:],
                                mybir.ActivationFunctionType.Identity,
                                bias=b2_t[:, 0:1],
                                scale=dinvcol_t[:, b:b + 1])
                            nc.scalar.dma_start(out[b * P:(b + 1) * P, 0:1], osb[:])

    nc.compile()
    return nc


def prepare_inputs(x, edge_index, W1, b1, W2, b2, n_cores, plan=None,
                   lo_split=32768):
    n_nodes = x.shape[0]
    if plan is None:
        plan = make_plan(np.asarray(edge_index), n_nodes, n_cores,
                         lo_split=lo_split)
    dinv = plan["dinv"]
    x_bf = (np.asarray(x, np.float32) * dinv[:, None]).astype(ml_dtypes.bfloat16)
    w1_bf = np.asarray(W1, np.float32).astype(ml_dtypes.bfloat16)
    w2_bf = np.asarray(W2, np.float32).astype(ml_dtypes.bfloat16)
    b1c = np.asarray(b1, np.float32).reshape(P, 1)
    b2c = np.full((P, 1), np.asarray(b2, np.float32).reshape(-1)[0], np.float32)
    in_maps = []
    for c in range(n_cores):
        in_maps.append(dict(
            xt=x_bf, idxA=plan["idxA"][c], idxD=plan["idxD"][c],
            Sd=plan["SA"][c], S2d=plan["SD"][c],
            dinvrep=plan["dinvrep"][c], dinvcol=plan["dinv_cols"][c],
            dinvpos=plan["dinv_pos"].reshape(-1, 1),
            w1=w1_bf, b1=b1c, w2=w2_bf, b2=b2c,
        ))
    return plan, in_maps


def assemble_output(results, plan, n_nodes, n_cores):
    n_own = plan["n_own"]
    pos = plan["blk_of"] * P + plan["dloc_of"]      # [n_cores, n_own]
    outs = [results[c]["out"][pos[c], 0] for c in range(n_cores)]
    return np.concatenate(outs, axis=0)[:n_nodes].astype(np.float32).reshape(-1, 1)


# ======================================================================
# Self-contained kernel frontend (harness entry point)
# ======================================================================
from concourse import bass_utils as _bass_utils

N_NODES = 50000
N_CORES = 8
_kernel_cache = {}


def _plan_signature(plan):
    return (plan["tot"], plan["tot_d"], tuple(plan["instrs_a"]),
            tuple(plan["instrs_d"]),
            tuple((tuple(ch["blocks"]), ch["lo"], ch["hi"],
                   tuple(sorted(ch["lo_runs"].items())),
                   tuple(sorted(ch["hi_runs"].items())))
                  for ch in plan["chunks"]),
            tuple((tuple(ch["blocks"]), ch["lo"], ch["hi"],
                   tuple(sorted(ch["lo_runs"].items())),
                   tuple(sorted(ch["hi_runs"].items())))
                  for ch in plan["chunks_d"]))


def kernel(x, edge_index, W1, b1, W2, b2):
    """Full-input GCN forward on 8 NeuronCores; returns [N, 1] float32."""
    x = np.asarray(x)
    edge_index = np.asarray(edge_index)
    n_nodes = x.shape[0]
    plan = make_plan(edge_index, n_nodes, N_CORES,
                     lo_split=min(32768, n_nodes))
    plan_d, in_maps = prepare_inputs(x, edge_index, W1, b1, W2, b2, N_CORES,
                                     plan=plan)
    sig = _plan_signature(plan)
    nc = _kernel_cache.get(sig)
    if nc is None:
        nc = build_kernel(plan, n_nodes, N_CORES)
        _kernel_cache[sig] = nc
    res = _bass_utils.run_bass_kernel_spmd(
        nc, in_maps, core_ids=list(range(N_CORES)))
    return assemble_output(res.results, plan, n_nodes, N_CORES)



# revision 5
# speedup vs baseline: 1.3396x; 1.3396x over previous
"""2-layer GCN on 8 TRN2 cores — v4.

v4 changes vs v2 (762us baseline):
  - Phase A one-hot S is built ON-CHIP (DVE is_equal against an iota row,
    driven by compact dst-local codes) instead of streaming 27MB from HBM.
  - Phase D is restructured: each dst is assigned to a (block, lane) slot
    with degree-sorted blocks (uniform padded width per block); h2 values
    are gathered per-edge straight into that grid with a paired dma_gather
    (elem_size=128, elem_step=64 -> 512B descriptors at full DMA bus rate,
    int16 pair indices need no lo/hi split), then parity-selected and
    block-reduced on DVE. No S2 stream, no phase-D matmuls.
"""
import math
import numpy as np
import ml_dtypes

import concourse.bass as bass
import concourse.bacc as bacc
import concourse.tile as tile
import concourse.mybir as mybir
from concourse import library_config

P = 128
REPW = 64         # h2rep row width (f32 -> 256B)
NBLK = 4          # dst blocks per chunk (phase A)
MAX_DESC = 1024   # single-packet dma_gather limit
DCOLS = 8         # phase D gather columns per instruction (8*128 = 1024 desc)
DCHUNK = 64       # phase D grid columns per R tile (extraction granularity)


def _build_side(idx_val, is_lo, core2, blk2, dloc2, n_cores, nb):
    """Slot/chunk/instruction structure for the phase-A gather side.

    idx_val: rebased gather index per edge (valid for its group's table
    slice). is_lo: group flag. All arrays sorted by (core, blk, ~is_lo).
    """
    counts = np.zeros((n_cores, nb, 2), np.int64)
    np.add.at(counts, (core2, blk2, (~is_lo).astype(np.int64)), 1)
    kL = np.ceil(counts[:, :, 0] / P).astype(np.int64).max(axis=0)
    kH = np.ceil(counts[:, :, 1] / P).astype(np.int64).max(axis=0)

    chunks = []
    slot_ptr = 0
    for c0 in range(0, nb, NBLK):
        blocks = list(range(c0, min(c0 + NBLK, nb)))
        lo_runs = {}
        lo_start = slot_ptr
        for b in blocks:
            lo_runs[b] = (slot_ptr, slot_ptr + int(kL[b]))
            slot_ptr += int(kL[b])
        lo_end = slot_ptr
        hi_runs = {}
        for b in blocks:
            hi_runs[b] = (slot_ptr, slot_ptr + int(kH[b]))
            slot_ptr += int(kH[b])
        chunks.append(dict(blocks=blocks, lo=(lo_start, lo_end),
                           hi=(lo_end, slot_ptr), lo_runs=lo_runs,
                           hi_runs=hi_runs))
    tot = slot_ptr

    idx_f = np.zeros((n_cores, tot * P), np.int16)
    # dst_local per edge slot-position; 255 = padding (matches no one-hot col)
    dl_f = np.full((n_cores, tot * P), 255.0, np.float32)

    key = (core2 * nb + blk2) * 2 + (~is_lo).astype(np.int64)
    run_start = np.searchsorted(key, np.arange(n_cores * nb * 2))
    nE = len(core2)
    for c in range(n_cores):
        for b in range(nb):
            ch = chunks[b // NBLK]
            for g, runs in ((0, ch["lo_runs"]), (1, ch["hi_runs"])):
                kidx = (c * nb + b) * 2 + g
                a = run_start[kidx]
                e = run_start[kidx + 1] if kidx + 1 < len(run_start) else nE
                n = e - a
                if n == 0:
                    continue
                s0, s1 = runs[b]
                assert n <= (s1 - s0) * P
                # sort the run's edges by gather index for DRAM locality
                perm = np.argsort(idx_val[a:e], kind="stable")
                fl = np.arange(n) + s0 * P
                idx_f[c, fl] = idx_val[a:e][perm].astype(np.int16)
                dl_f[c, fl] = dloc2[a:e][perm]

    def wrap(flat_i16, s0, s1):
        seg = flat_i16[s0 * P:s1 * P]
        return np.tile(seg.reshape(16, -1, order="F"), (8, 1))

    instrs0 = []
    for ch in chunks:
        if ch["lo"][1] > ch["lo"][0]:
            instrs0.append(ch["lo"])
        if ch["hi"][1] > ch["hi"][0]:
            instrs0.append(ch["hi"])
    instrs = []
    for s0, s1 in instrs0:
        n = s1 - s0
        parts = math.ceil(n * P / MAX_DESC)
        step = math.ceil(n / parts)
        for a in range(s0, s1, step):
            instrs.append((a, min(a + step, s1)))

    idx = [np.concatenate([wrap(idx_f[c], s0, s1) for (s0, s1) in instrs], axis=1)
           for c in range(n_cores)]
    # dst-local codes [P, tot] bf16 (255 = pad -> no one-hot match); the
    # one-hot S is expanded on-chip by DVE is_equal against an iota row.
    dlocT = [np.ascontiguousarray(
        dl_f[c].reshape(tot, P).T.astype(ml_dtypes.bfloat16))
        for c in range(n_cores)]
    return dict(chunks=chunks, tot=tot, instrs=instrs, idx=idx, dlocT=dlocT)


def make_plan(edge_index, n_nodes, n_cores, lo_split=32768):
    src = np.concatenate([edge_index[0], np.arange(n_nodes)]).astype(np.int64)
    dst = np.concatenate([edge_index[1], np.arange(n_nodes)]).astype(np.int64)
    deg = np.bincount(dst, minlength=n_nodes).astype(np.float32)
    dinv = 1.0 / np.sqrt(np.maximum(deg, 1.0))

    n_own = n_nodes // n_cores
    assert n_own * n_cores == n_nodes
    nb = math.ceil(n_own / P)
    n_pad = nb * P

    core = (dst // n_own).astype(np.int64)
    loc = dst - core * n_own

    # balanced dst->block assignment (snake-deal by degree) for phase A
    blk_of = np.zeros((n_cores, n_own), np.int64)
    dloc_of = np.zeros((n_cores, n_own), np.int64)
    degc = np.zeros((n_cores, n_own), np.int64)
    np.add.at(degc, (core, loc), 1)
    snake = np.concatenate([np.arange(nb), np.arange(nb)[::-1]])
    for c in range(n_cores):
        order = np.argsort(-degc[c], kind="stable")
        bseq = snake[np.arange(n_own) % (2 * nb)]
        blk_of[c, order] = bseq
        pos = np.zeros(nb, np.int64)
        dl = np.zeros(n_own, np.int64)
        for i, d in enumerate(order):
            b = bseq[i]
            dl[d] = pos[b]
            pos[b] += 1
        assert pos.max() <= P
        dloc_of[c] = dl
    blk = blk_of[core, loc]
    dloc = dloc_of[core, loc]

    # h2all position of each node (phase D gather index space)
    src_core = src // n_own
    src_loc = src - src_core * n_own
    h2pos = (src_core * n_pad + blk_of[src_core, src_loc] * P
             + dloc_of[src_core, src_loc])

    # ---------- phase A side (split by src id) ----------
    is_loA = src < lo_split
    orderA = np.lexsort((~is_loA, blk, core))
    vA = np.where(is_loA, src, src - lo_split)[orderA]
    sideA = _build_side(vA, is_loA[orderA], core[orderA], blk[orderA],
                        dloc[orderA], n_cores, nb)

    # ---------- phase D grid (degree-sorted blocks, uniform pad) ----------
    blk2_of = np.zeros((n_cores, n_own), np.int64)
    dloc2_of = np.zeros((n_cores, n_own), np.int64)
    for c in range(n_cores):
        order2 = np.argsort(-degc[c], kind="stable")
        blk2_of[c, order2] = np.arange(n_own) // P
        dloc2_of[c, order2] = np.arange(n_own) % P
    # global per-block max degree -> uniform column widths across cores
    dmax = np.zeros((n_cores, nb), np.int64)
    for c in range(n_cores):
        np.maximum.at(dmax[c], blk2_of[c], degc[c])
    Dhat = dmax.max(axis=0)                       # [nb]
    off2 = np.concatenate([[0], np.cumsum(Dhat)])  # [nb+1]
    lhat = int(off2[-1])

    # per-edge grid slot: (lane = dloc2, col = off2[blk2] + rank within dst)
    idxD2 = []
    pa = []
    pb = []
    e_core = core
    pair = (h2pos // 2).astype(np.int16)
    par = (h2pos % 2).astype(np.float32)
    for c in range(n_cores):
        sel = np.nonzero(e_core == c)[0]
        d_loc = loc[sel]
        lane = dloc2_of[c, d_loc]
        colb = off2[blk2_of[c, d_loc]]
        # rank within dst (stable order): count occurrences
        orderE = np.argsort(d_loc, kind="stable")
        sel_s = sel[orderE]
        d_s = d_loc[orderE]
        rank = np.arange(len(sel)) - np.searchsorted(d_s, d_s)
        lane_s = lane[orderE]
        col_s = colb[orderE] + rank
        assert (rank < Dhat[blk2_of[c, d_s]]).all()
        flat = col_s * P + lane_s              # descriptor index
        idx_full = np.zeros(lhat * P, np.int16)
        a_full = np.zeros(lhat * P, np.float32)
        b_full = np.zeros(lhat * P, np.float32)
        idx_full[flat] = pair[sel_s]
        a_full[flat] = 1.0 - par[sel_s]
        b_full[flat] = par[sel_s]
        idxD2.append(np.tile(idx_full.reshape(16, -1, order="F"), (8, 1)))
        pa.append(np.ascontiguousarray(
            a_full.reshape(lhat, P).T.astype(ml_dtypes.bfloat16)))
        pb.append(np.ascontiguousarray(
            b_full.reshape(lhat, P).T.astype(ml_dtypes.bfloat16)))

    # dinv by phase-D grid position [P, nb] per core (0 at unused slots)
    n_rep = n_cores * n_pad
    dinv_pos = np.zeros(n_rep, np.float32)        # dinv by h2all position
    allpos = (np.repeat(np.arange(n_cores), n_own) * n_pad
              + (blk_of * P + dloc_of).reshape(-1))
    dinv_pos[allpos] = dinv[np.arange(n_nodes)]
    dinvrep = []                                  # [P, n_pad] row-replicated
    dinv_col2 = []                                # [P, nb] phase-D grid dinv
    for c in range(n_cores):
        dp = dinv_pos[c * n_pad:(c + 1) * n_pad]
        dinvrep.append(np.tile(dp, (P, 1)).astype(np.float32))
        dc2 = np.zeros((P, nb), np.float32)
        dc2[dloc2_of[c], blk2_of[c]] = dinv[c * n_own + np.arange(n_own)]
        dinv_col2.append(np.ascontiguousarray(dc2))
    return dict(
        n_own=n_own, nb=nb, n_pad=n_pad, lo_split=lo_split,
        blk_of=blk_of, dloc_of=dloc_of, blk2_of=blk2_of, dloc2_of=dloc2_of,
        dinv=dinv, dinv_pos=dinv_pos, dinvrep=dinvrep, dinv_col2=dinv_col2,
        Dhat=Dhat, off2=off2, lhat=lhat,
        tot=sideA["tot"], chunks=sideA["chunks"], instrs_a=sideA["instrs"],
        idxA=sideA["idx"], dlocA=sideA["dlocT"],
        idxD2=idxD2, pa=pa, pb=pb,
    )


def build_kernel(plan, n_nodes, n_cores, npass=1, phases="abcd"):
    nb, tot, n_pad = plan["nb"], plan["tot"], plan["n_pad"]
    lhat = plan["lhat"]
    off2 = plan["off2"]
    Dhat = plan["Dhat"]
    lo_split = plan["lo_split"]
    n_rep = n_cores * n_pad               # h2rep rows (h2all positions)
    RP = next(rp for rp in (128, 125, 100, 64, 50, 25, 16, 8)
              if n_rep % rp == 0)
    RK = n_rep // RP                      # h2rep SBUF staging shape

    nc = bacc.Bacc("TRN2", target_bir_lowering=False, debug=False,
                   enable_asserts=False, num_devices=n_cores, num_swdge_queues=4)
    f32, bf16, i16 = mybir.dt.float32, mybir.dt.bfloat16, mybir.dt.int16

    xt = nc.dram_tensor("xt", [n_nodes, P], bf16, kind="ExternalInput").ap()
    idxA = nc.dram_tensor("idxA", [P, tot * 8], i16, kind="ExternalInput").ap()
    idxD = nc.dram_tensor("idxD", [P, lhat * 8], i16, kind="ExternalInput").ap()
    dlocA = nc.dram_tensor("dlocA", [P, tot], bf16, kind="ExternalInput").ap()
    pa_d = nc.dram_tensor("pa", [P, lhat], bf16, kind="ExternalInput").ap()
    pb_d = nc.dram_tensor("pb", [P, lhat], bf16, kind="ExternalInput").ap()
    iota_d = nc.dram_tensor("iota", [P, P], bf16, kind="ExternalInput").ap()
    dinvrep = nc.dram_tensor("dinvrep", [P, n_pad], f32, kind="ExternalInput").ap()
    dinvcol2 = nc.dram_tensor("dinvcol2", [P, nb], f32, kind="ExternalInput").ap()
    dinvpos = nc.dram_tensor("dinvpos", [n_cores * n_pad, 1], f32,
                             kind="ExternalInput").ap()
    w1 = nc.dram_tensor("w1", [P, P], bf16, kind="ExternalInput").ap()
    b1 = nc.dram_tensor("b1", [P, 1], f32, kind="ExternalInput").ap()
    w2 = nc.dram_tensor("w2", [P, 1], bf16, kind="ExternalInput").ap()
    b2 = nc.dram_tensor("b2", [P, 1], f32, kind="ExternalInput").ap()
    out = nc.dram_tensor("out", [n_pad, 1], f32, kind="ExternalOutput").ap()

    qn = [0]

    def next_q():
        qn[0] = (qn[0] + 1) % 4
        return qn[0]

    with tile.TileContext(nc) as tc:
        with (
            tc.tile_pool(name="const", bufs=1) as cpool,
            tc.tile_pool(name="dram", bufs=max(npass, 1), space="DRAM") as dpool,
        ):
            nc.gpsimd.load_library(library_config.mlp)
            idxA_t = cpool.tile([P, tot * 8], i16)
            idxD_t = cpool.tile([P, lhat * 8], i16)
            dlocA_t = cpool.tile([P, tot], bf16)
            pa_t = cpool.tile([P, lhat], bf16)
            pb_t = cpool.tile([P, lhat], bf16)
            iota_t = cpool.tile([P, P], bf16)
            dinvrep_t = cpool.tile([P, n_pad], f32)
            dinvcol2_t = cpool.tile([P, nb], f32)
            w1_t = cpool.tile([P, P], bf16)
            b1_t = cpool.tile([P, 1], f32)
            w2_t = cpool.tile([P, 1], bf16)
            b2_t = cpool.tile([P, 1], f32)
            h2strip = cpool.tile([1, n_pad], f32)
            for t, d in ((idxA_t, idxA), (idxD_t, idxD), (dlocA_t, dlocA),
                         (pa_t, pa_d), (pb_t, pb_d), (iota_t, iota_d),
                         (dinvrep_t, dinvrep), (dinvcol2_t, dinvcol2),
                         (w1_t, w1), (b1_t, b1), (w2_t, w2), (b2_t, b2)):
                nc.sync.dma_start(t[:], d[:])

            for _ps in range(npass):
                if _ps:
                    tc.strict_bb_all_engine_barrier()
                h2loc = dpool.tile([1, n_pad], f32, name=f"h2loc{_ps}")
                h2all = dpool.tile([n_cores, n_pad], f32, addr_space="Shared",
                                   name=f"h2all{_ps}")
                h2rep = dpool.tile([n_rep, REPW], f32, name=f"h2rep{_ps}")

                # ---------------- phase A + B ----------------
                with (
                    tc.tile_pool(name="gbuf", bufs=2) as gpool,
                    tc.tile_pool(name="spool", bufs=2) as spool,
                    tc.tile_pool(name="small_ab", bufs=2) as smpool,
                    tc.tile_pool(name="agg_ps", bufs=3, space="PSUM") as agg_ps,
                    tc.tile_pool(name="o1_ps", bufs=2, space="PSUM") as o1_ps,
                    tc.tile_pool(name="h2_ps", bufs=2, space="PSUM") as h2_ps,
                ):
                    ia = 0
                    icol = 0
                    for ch in plan["chunks"]:
                        c0, c1 = ch["lo"][0], ch["hi"][1]
                        k = c1 - c0
                        G = gpool.tile([P, k * P], bf16, tag="G", name="G")
                        while ia < len(plan["instrs_a"]) and plan["instrs_a"][ia][0] < c1:
                            s0, s1 = plan["instrs_a"][ia]
                            n = s1 - s0
                            src_tab = (xt[:lo_split, :] if s0 < ch["hi"][0]
                                       else xt[lo_split:, :])
                            nc.gpsimd.dma_gather(
                                G[:, (s0 - c0) * P:(s1 - c0) * P].rearrange(
                                    "p (c d) -> p c d", d=P),
                                src_tab,
                                idxA_t[:, icol:icol + n * 8],
                                n * P, n * P, P,
                                single_packet=True, queue_num=next_q(),
                            )
                            icol += n * 8
                            ia += 1

                        S = spool.tile([P, k * P], bf16, tag="S", name="S")
                        nc.vector.tensor_tensor(
                            out=S[:].rearrange("p (c j) -> p c j", j=P),
                            in0=dlocA_t[:, c0:c1].rearrange(
                                "p (c a) -> p c a", a=1).to_broadcast([P, k, P]),
                            in1=iota_t[:].rearrange(
                                "p (a j) -> p a j", a=1).to_broadcast([P, k, P]),
                            op=mybir.AluOpType.is_equal)

                        for b in ch["blocks"]:
                            slots = (list(range(*ch["lo_runs"][b]))
                                     + list(range(*ch["hi_runs"][b])))
                            aggT = agg_ps.tile([P, P], f32, tag="agg", name="aggT")
                            for i, s in enumerate(slots):
                                sl = slice((s - c0) * P, (s - c0 + 1) * P)
                                nc.tensor.matmul(
                                    out=aggT[:], lhsT=G[:, sl], rhs=S[:, sl],
                                    start=(i == 0), stop=(i == len(slots) - 1))
                            aggT_sb = smpool.tile([P, P], bf16, tag="aggsb",
                                                  name="aggT_sb")
                            nc.vector.tensor_tensor(
                                out=aggT_sb[:], in0=aggT[:],
                                in1=dinvrep_t[:, b * P:(b + 1) * P],
                                op=mybir.AluOpType.mult)
                            o1 = o1_ps.tile([P, P], f32, tag="o1", name="o1")
                            nc.tensor.matmul(out=o1[:], lhsT=w1_t[:], rhs=aggT_sb[:],
                                             start=True, stop=True)
                            r1 = smpool.tile([P, P], bf16, tag="r1", name="r1")
                            nc.scalar.activation(r1[:], o1[:],
                                                 mybir.ActivationFunctionType.Relu,
                                                 bias=b1_t[:, 0:1])
                            h2p = h2_ps.tile([1, P], f32, tag="h2", name="h2p")
                            nc.tensor.matmul(out=h2p[:], lhsT=w2_t[:], rhs=r1[:],
                                             start=True, stop=True)
                            nc.vector.tensor_copy(
                                h2strip[0:1, b * P:(b + 1) * P], h2p[:])

                # ---------------- phase C ----------------
                if "c" not in phases:
                    nc.sync.dma_start(out[0:P, 0:1],
                                      h2strip[0:1, 0:P].rearrange("a b -> b a"))
                    continue
                tc.strict_bb_all_engine_barrier()
                nc.sync.dma_start(h2loc[:], h2strip[:])
                nc.gpsimd.collective_compute(
                    "AllGather", mybir.AluOpType.bypass,
                    ins=[h2loc.opt()], outs=[h2all.opt()],
                    replica_groups=[list(range(n_cores))],
                )
                # build h2rep [n_rep, REPW] (h2 * dinv_src, row-replicated)
                with tc.tile_pool(name="rep", bufs=1) as rpool:
                    hin = rpool.tile([RP, RK], f32)
                    nc.sync.dma_start(
                        hin[:], h2all[:].rearrange("a b -> (a b)").rearrange(
                            "(p k) -> p k", p=RP))
                    dv = rpool.tile([RP, RK], f32)
                    nc.sync.dma_start(
                        dv[:], dinvpos[:, 0:1].rearrange("(p k) a -> p (k a)", p=RP))
                    hsc = rpool.tile([RP, RK], f32)
                    nc.vector.tensor_tensor(out=hsc[:], in0=hin[:], in1=dv[:],
                                            op=mybir.AluOpType.mult)
                    hx = rpool.tile([RP, RK * REPW], f32)
                    nc.vector.tensor_copy(
                        hx[:].rearrange("p (k d) -> p k d", d=REPW),
                        hsc[:].rearrange("p (k a) -> p k a", a=1).to_broadcast(
                            [RP, RK, REPW]))
                    nc.sync.dma_start(
                        h2rep[:].rearrange("(p k) d -> p (k d)", p=RP), hx[:])

                # ---------------- phase D ----------------
                if "d" not in phases:
                    nc.sync.dma_start(out[0:P, 0:1],
                                      h2strip[0:1, 0:P].rearrange("a b -> b a"))
                    continue
                rep_pairs = h2rep[:].rearrange("(a b) d -> a (b d)", b=2)
                with (
                    tc.tile_pool(name="repg", bufs=2) as repg,
                    tc.tile_pool(name="dgrid", bufs=1) as dgrid,
                ):
                    vals = dgrid.tile([P, lhat], f32)
                    outg = dgrid.tile([P, nb], f32)
                    icol = 0
                    for c0 in range(0, lhat, DCHUNK):
                        c1 = min(c0 + DCHUNK, lhat)
                        k = c1 - c0
                        R = repg.tile([P, DCHUNK * 2 * REPW], f32, tag="R",
                                      name="R")
                        for s0 in range(c0, c1, DCOLS):
                            s1 = min(s0 + DCOLS, c1)
                            n = s1 - s0
                            nc.gpsimd.dma_gather(
                                R[:, (s0 - c0) * 2 * REPW:(s1 - c0) * 2 * REPW]
                                .rearrange("p (c d) -> p c d", d=2 * REPW),
                                rep_pairs,
                                idxD_t[:, icol:icol + n * 8],
                                n * P, n * P, 2 * REPW,
                                single_packet=True, queue_num=next_q(),
                            )
                            icol += n * 8
                        Rv = R[:].rearrange("p (c d) -> p c d", d=2 * REPW)
                        t0 = dgrid.tile([P, DCHUNK], f32, tag="t0", name="t0")
                        nc.vector.tensor_tensor(
                            out=t0[:, :k].rearrange("p (c a) -> p c a", a=1),
                            in0=Rv[:, :k, 0:1],
                            in1=pa_t[:, c0:c1].rearrange("p (c a) -> p c a", a=1),
                            op=mybir.AluOpType.mult)
                        t1 = dgrid.tile([P, DCHUNK], f32, tag="t1", name="t1")
                        nc.vector.tensor_tensor(
                            out=t1[:, :k].rearrange("p (c a) -> p c a", a=1),
                            in0=Rv[:, :k, REPW:REPW + 1],
                            in1=pb_t[:, c0:c1].rearrange("p (c a) -> p c a", a=1),
                            op=mybir.AluOpType.mult)
                        nc.vector.tensor_tensor(
                            out=vals[:, c0:c1], in0=t0[:, :k], in1=t1[:, :k],
                            op=mybir.AluOpType.add)
                    for b in range(nb):
                        nc.vector.tensor_reduce(
                            out=outg[:, b:b + 1],
                            in_=vals[:, int(off2[b]):int(off2[b + 1])],
                            axis=mybir.AxisListType.X,
                            op=mybir.AluOpType.add)
                    outv = dgrid.tile([P, nb], f32)
                    nc.vector.tensor_tensor(out=outv[:], in0=outg[:],
                                            in1=dinvcol2_t[:],
                                            op=mybir.AluOpType.mult)
                    outf = dgrid.tile([P, nb], f32)
                    nc.vector.tensor_tensor(
                        out=outf[:], in0=outv[:],
                        in1=b2_t[:, 0:1].to_broadcast([P, nb]),
                        op=mybir.AluOpType.add)
                    nc.scalar.dma_start(out[:, 0:1].rearrange(
                        "(b p) a -> p (b a)", p=P), outf[:])

    nc.compile()
    return nc


def prepare_inputs(x, edge_index, W1, b1, W2, b2, n_cores, plan=None,
                   lo_split=32768):
    n_nodes = x.shape[0]
    if plan is None:
        plan = make_plan(np.asarray(edge_index), n_nodes, n_cores,
                         lo_split=lo_split)
    dinv = plan["dinv"]
    x_bf = (np.asarray(x, np.float32) * dinv[:, None]).astype(ml_dtypes.bfloat16)
    w1_bf = np.asarray(W1, np.float32).astype(ml_dtypes.bfloat16)
    w2_bf = np.asarray(W2, np.float32).astype(ml_dtypes.bfloat16)
    b1c = np.asarray(b1, np.float32).reshape(P, 1)
    b2c = np.full((P, 1), np.asarray(b2, np.float32).reshape(-1)[0], np.float32)
    iota = np.tile(np.arange(P, dtype=np.float32)[None, :], (P, 1)).astype(
        ml_dtypes.bfloat16)
    in_maps = []
    for c in range(n_cores):
        in_maps.append(dict(
            xt=x_bf, idxA=plan["idxA"][c], idxD=plan["idxD2"][c],
            dlocA=plan["dlocA"][c], pa=plan["pa"][c], pb=plan["pb"][c],
            iota=iota,
            dinvrep=plan["dinvrep"][c], dinvcol2=plan["dinv_col2"][c],
            dinvpos=plan["dinv_pos"].reshape(-1, 1),
            w1=w1_bf, b1=b1c, w2=w2_bf, b2=b2c,
        ))
    return plan, in_maps


def assemble_output(results, plan, n_nodes, n_cores):
    n_own = plan["n_own"]
    pos = plan["blk2_of"] * P + plan["dloc2_of"]      # [n_cores, n_own]
    outs = [results[c]["out"][pos[c], 0] for c in range(n_cores)]
    return np.concatenate(outs, axis=0)[:n_nodes].astype(np.float32).reshape(-1, 1)


# ======================================================================
# Self-contained kernel frontend (harness entry point)
# ======================================================================
from concourse import bass_utils as _bass_utils

N_NODES = 50000
N_CORES = 8
_kernel_cache = {}


def _plan_signature(plan):
    return (plan["tot"], plan["lhat"], tuple(plan["instrs_a"]),
            tuple(plan["Dhat"].tolist()),
            tuple((tuple(ch["blocks"]), ch["lo"], ch["hi"],
                   tuple(sorted(ch["lo_runs"].items())),
                   tuple(sorted(ch["hi_runs"].items())))
                  for ch in plan["chunks"]))


def kernel(x, edge_index, W1, b1, W2, b2):
    """Full-input GCN forward on 8 NeuronCores; returns [N, 1] float32."""
    x = np.asarray(x)
    edge_index = np.asarray(edge_index)
    n_nodes = x.shape[0]
    plan = make_plan(edge_index, n_nodes, N_CORES,
                     lo_split=min(32768, n_nodes))
    plan_d, in_maps = prepare_inputs(x, edge_index, W1, b1, W2, b2, N_CORES,
                                     plan=plan)
    sig = _plan_signature(plan)
    nc = _kernel_cache.get(sig)
    if nc is None:
        nc = build_kernel(plan, n_nodes, N_CORES)
        _kernel_cache[sig] = nc
    res = _bass_utils.run_bass_kernel_spmd(
        nc, in_maps, core_ids=list(range(N_CORES)))
    return assemble_output(res.results, plan, n_nodes, N_CORES)
